# revision 59
# baseline (speedup 1.0000x reference)
"""Two-layer GAT (DGL GATConv) on 8 Trainium2 NeuronCores.

Edge-parallel by dst node with degree-balanced dst windows and a shared
gather schedule for both layers:
  * Core c owns dst nodes [6250c, 6250(c+1)).  Its 6250 nodes are packed
    into 49 windows of 128 so each window's total in-degree fits K_g*128
    slots (K_g = 17 for the first NG17 windows, 15 after) -- ~1% padding
    vs per-half ceils.  Window order/content is a host-side permutation;
    the output is un-permuted on the host.
  * Table rows follow the AllGather chunk layout.  Slots are sorted by
    src table row; the first CS_g chunks are gathered from table[0:32768]
    (idx = row) and the rest from table[18432:] (idx = row - 18432), so
    int16 gather indices reach all 51200 rows with no per-half padding.
    The same index array serves layer 2 (chunks [0, CS_g) are rows
    < 30720 = the AllGather A sections, rest needs B).
  * Layer 1: x replicated; every core projects the FULL table locally in
    bf16 (rows 512B: fs bf16 264B = 128 feats + 4 el) -- no fp8 widen.
  * Layer 2: rows 256B (fs2 fp8 188B c-major | el2 bf16 8B), projected
    inline per group in the layer-1 epilogue and AllGathered in 4 chunks
    that overlap layer-1 edge compute; layer-2 runs as A-pass (chunks
    < CS_g) then B-pass accumulating through a bf16 segbuf.
  * Edge phase per group: dma_gather rows by src, one-hot scatter
    matmuls into PSUM.  er per edge comes from ohT.T @ erw accumulated
    with an identity matmul that adds the gathered el in the same PSUM
    tile; lrelu on DVE, exp on ACT, ee*fs on DVE (2x mode), scatter via
    oh_mc.T @ mt.  d-major feature order keeps broadcasts packed.
Softmax max-subtraction is skipped (|logits| small; alpha shift-invariant).
"""

import math
import os
import sys
from contextlib import ExitStack

import numpy as np

for _p in ("/opt/trn_rl_repo", "/root/.axon_site/_ro/trn_rl_repo"):
    if os.path.isdir(_p) and _p not in sys.path:
        sys.path.append(_p)

import ml_dtypes

import concourse.bass as bass
import concourse.tile as tile
from concourse import bacc, mybir
from concourse.bass_utils import run_bass_kernel_spmd

BF16 = ml_dtypes.bfloat16
FP8 = ml_dtypes.float8_e4m3

N = 50000
E = 800000
F_IN = 128
H, D, C = 4, 32, 47
HD = H * D            # 128
HC = H * C            # 188
NEG_SLOPE = 0.2

NCORES = 8
P = 128
NPC = N // NCORES         # 6250
HALF_A = 3840             # per-core shard low half (30 tiles of 128)
HALF_B = 2560             # per-core shard high half (20 tiles)
NPAD = HALF_A + HALF_B    # 6400 padded rows per core
S_A = NCORES * HALF_A     # 30720 rows in table section A
S_B = NCORES * HALF_B     # 20480 rows in table section B
NTAB = S_A + S_B          # 51200
G = math.ceil(NPC / P)    # 49 dst groups per core
NOUT = G * P              # 6272 output rows per core

W1 = HD + H               # 132 cols used in layer-1 rows (fs|el)
W2 = HC + H               # 192 (fs2|el2)
ELEM1 = 256               # layer-1 table row bytes (fp8 fs | bf16 el)
ELEM2 = 256               # layer-2 table row bytes (fp8 fs2 | bf16 el2)
OOR = 200.0               # out-of-window dst_local marker
HI_OFF = 18432            # hi-gather table base row
KMAX = 17
K15 = 15
KL2 = 10                  # max chunks per layer-2 pass (= max cs)
CROW = 196                # compact collective row bytes (fs2 fp8|el2 bf16)
S_LO = 32768              # lo-gather table rows

LAST_EXEC_NS = None
_LAST_SCHED = None
_LAST_NC = None

# AllGather chunk boundaries in own-tile units.  The first four cover
# rows 0..33792 > 32768 so the layer-2 A-pass (rows < 32768) can start
# once they have landed; small early chunks keep the collective chain
# abreast of layer-1 edge compute.
AG_CHUNKS = [(0, 9), (9, 17), (17, 25), (25, 33), (33, 41), (41, 50)]
N_ACHUNK = 4              # chunks needed by the layer-2 A-pass


def _row_of(core, pos):
    """Table row of (owning core, padded local position)."""
    conds, vals = [], []
    base = 0
    for (t0, t1) in AG_CHUNKS:
        lo, hi, rpc = t0 * P, t1 * P, (t1 - t0) * P
        conds.append((pos >= lo) & (pos < hi))
        vals.append(base + core * rpc + (pos - lo))
        base += NCORES * rpc
    return np.select(conds, vals)


def _cs_of(k):
    # lo/hi gather split chunk: lo slots hold rows < 32768 (also the
    # layer-2 A-pass, gated on the first three AllGather chunks)
    return 10 if k == KMAX else 8


def _pack_core(deg, ng17):
    """Greedy-pack NPC nodes (by in-degree) into G bins of 128.

    Returns bins [G, 128] of local node ids (-1 = virtual pad)."""
    targets = np.where(np.arange(G) < ng17, KMAX * P, K15 * P)
    order = np.argsort(-deg, kind="stable")
    sums = np.zeros(G, np.int64)
    cards = np.zeros(G, np.int64)
    bins = np.full((G, P), -1, np.int64)
    for v in order:
        d = deg[v]
        # bin whose remaining slots need the highest average degree
        need = (targets - sums - d) / np.maximum(P - cards, 1)
        ok = (cards < P) & (targets - sums >= d)
        if ok.any():
            g = int(np.where(ok, need, -np.inf).argmax())
        else:
            g = int(np.where(cards < P, targets - sums, -np.inf).argmax())
        bins[g, cards[g]] = v
        cards[g] += 1
        sums[g] += d
    if not (sums <= targets).all():
        return None
    return bins


def _wrap(a):
    # [n] gather idx list -> [128, n/16] wrapped+replicated int16
    n = a.shape[-1]
    w = a.reshape(n // 16, 16).T            # [16, n/16]
    return np.tile(w, (8, 1)).astype(np.int16)


def _schedule(src, dst):
    """Degree-balanced windows + straddled lo/hi chunk schedule.

    Returns per-core packed arrays + the shared K profile."""
    src = src.astype(np.int64)
    dst = dst.astype(np.int64)
    core_of = dst // NPC
    dloc = dst % NPC

    deg = np.zeros((NCORES, NPC), np.int64)
    np.add.at(deg, (core_of, dloc), 1)
    tot = deg.sum(1)
    ng17 = int(max(2, math.ceil((tot.max() - G * K15 * P) / ((KMAX - K15) * P))))
    ng17 = min(G, ng17 + 2)   # slack for card-128 capacity stranding
    while True:
        packed = [_pack_core(deg[c], ng17) for c in range(NCORES)]
        if all(b is not None for b in packed):
            break
        assert ng17 < G, "packing failed even with uniform KMAX"
        ng17 = min(G, ng17 + 4)
    # small groups first so the first AllGather fires earlier
    kprof = [K15 if g < G - ng17 else KMAX for g in range(G)]
    bins = np.stack([b[::-1] for b in packed])

    # local node -> (group, window pos); virtual pads keep -1
    g_of = np.zeros((NCORES, NPC), np.int64)
    w_of = np.zeros((NCORES, NPC), np.int64)
    for c in range(NCORES):
        flat = bins[c].ravel()
        real = flat >= 0
        g_of[c, flat[real]] = np.nonzero(real)[0] // P
        w_of[c, flat[real]] = np.nonzero(real)[0] % P

    # permuted padded position of every global node (for table rows)
    pos_of = np.zeros(N, np.int64)
    for c in range(NCORES):
        pos_of[c * NPC:(c + 1) * NPC] = g_of[c] * P + w_of[c]
    rows_all = _row_of(np.arange(N) // NPC, pos_of)

    rowsrc = rows_all[src]
    eg = g_of[core_of, dloc]
    ew = w_of[core_of, dloc]

    KW = KMAX * 8
    idx_w = np.zeros((NCORES, G, P, KW), np.int16)
    dstl_col = np.full((NCORES, G, P, KMAX), OOR, np.float32)
    dstl_row = np.full((NCORES, G, KMAX * P), OOR, np.float32)

    for c in range(NCORES):
        sel = core_of == c
        r, g, w = rowsrc[sel], eg[sel], ew[sel]
        order = np.lexsort((r, g))
        r, g, w = r[order], g[order], w[order]
        starts = np.searchsorted(g, np.arange(G + 1))
        for gg in range(G):
            k = kprof[gg]
            cs = _cs_of(k)
            lo_slots, slots = cs * P, k * P
            rg = r[starts[gg]:starts[gg + 1]]
            wg = w[starts[gg]:starts[gg + 1]]
            cnt = len(rg)
            assert cnt <= slots, (c, gg, cnt, slots)
            n_lo = min(int(np.searchsorted(rg, S_LO)), lo_slots)
            assert int(np.searchsorted(rg, HI_OFF)) <= n_lo, (c, gg)
            assert cnt - n_lo <= slots - lo_slots, (c, gg, cnt, n_lo)
            idx = np.zeros(slots, np.int64)
            dl = np.full(slots, OOR, np.float32)
            idx[:n_lo] = rg[:n_lo]
            dl[:n_lo] = wg[:n_lo]
            n_hi = cnt - n_lo
            idx[lo_slots:lo_slots + n_hi] = rg[n_lo:] - HI_OFF
            dl[lo_slots:lo_slots + n_hi] = wg[n_lo:]
            assert (idx >= 0).all() and (idx < 32768).all()
            idx_w[c, gg, :, :cs * 8] = _wrap(idx[:lo_slots])
            idx_w[c, gg, :, cs * 8:k * 8] = _wrap(idx[lo_slots:])
            dstl_col[c, gg, :, :k] = dl.reshape(k, P).T
            dstl_row[c, gg, :slots] = dl

    sched = np.concatenate(
        [idx_w, dstl_col.astype(BF16).view(np.int16)], axis=3)
    return dict(sched=sched, dstl_row=dstl_row.astype(BF16), kprof=kprof,
                bins=bins, rows_all=rows_all, pos_of=pos_of)


def _blockdiag(a, hd, h, dim):
    out = np.zeros((hd, h), dtype=np.float32)
    for i in range(h):
        out[i * dim:(i + 1) * dim, i] = a[i]
    return out


def _build_program(kprof):
    SCHED_W = KMAX * 8 + KMAX    # idx cols + dstl_col cols (int16)
    SB = 7                       # sched groups per DMA batch
    nc = bacc.Bacc("TRN2", target_bir_lowering=False, debug=False,
                   num_devices=NCORES, num_swdge_queues=2)
    dt = mybir.dt
    f32, bf16, i16, u8 = dt.float32, dt.bfloat16, dt.int16, dt.uint8
    fp8 = dt.float8e4

    def inp(name, shape, d=f32):
        return nc.dram_tensor(name, shape, d, kind="ExternalInput").ap()

    xT_all = inp("xT_all", [P, NTAB], bf16)      # replicated, table-row order
    xT_own = inp("xT_own", [P, NPAD], bf16)      # own nodes, window order
    w1cat = inp("w1cat", [F_IN, W1 + H], bf16)   # fs(d-major)|el|er
    w2cat = inp("w2cat", [F_IN, W2 + H], bf16)   # fs2(c-major)|el2|er2
    b1_t = inp("b1_t", [P, HD])
    b2m_t = inp("b2m_t", [P, C])
    iota_mc_in = inp("iota_mc", [P, P * KMAX], bf16)  # [p, m*KMAX+c] = m
    iota_c = inp("iota_c", [P, 1])               # iota_c[p, 0] = p (f32)
    ident_t = inp("ident_t", [P, P], bf16)
    sched_in = inp("sched_in", [G, P, SCHED_W], i16)
    dstl_row_in = inp("dstl_row_in", [G, KMAX * P], bf16)

    y_out = nc.dram_tensor("y_out", [NOUT, C], f32, kind="ExternalOutput").ap()

    tab1 = nc.dram_tensor("tab1", [NTAB, ELEM1], u8).ap()
    tab2_own = [
        nc.dram_tensor(f"tab2_own{k}", [(t1 - t0) * P, ELEM2], u8).ap()
        for k, (t0, t1) in enumerate(AG_CHUNKS)]
    tab2 = nc.dram_tensor("tab2", [NTAB, ELEM2], u8,
                          addr_space="Shared").ap()

    with tile.TileContext(nc) as tc, ExitStack() as ctx:
        const = ctx.enter_context(tc.tile_pool(name="const", bufs=1))
        sb = ctx.enter_context(tc.tile_pool(name="sb", bufs=3))
        gat = ctx.enter_context(tc.tile_pool(name="gat", bufs=3))
        ps = ctx.enter_context(tc.tile_pool(name="ps", bufs=2, space="PSUM"))
        psg = ctx.enter_context(tc.tile_pool(name="psg", bufs=3, space="PSUM"))
        big = ctx.enter_context(tc.tile_pool(name="big", bufs=1))

        iota_mc = const.tile([P, P * KMAX], bf16)
        nc.sync.dma_start(iota_mc[:], iota_mc_in[:])
        iotac = const.tile([P, 1], f32)
        nc.sync.dma_start(iotac[:], iota_c[:])
        ident = const.tile([P, P], bf16)
        nc.sync.dma_start(ident[:], ident_t[:])
        b1s = const.tile([P, HD], f32)
        nc.sync.dma_start(b1s[:], b1_t[:])
        b2ms = const.tile([P, C], f32)
        nc.sync.dma_start(b2ms[:], b2m_t[:])
        w1 = const.tile([P, W1 + H], bf16)
        nc.sync.dma_start(w1[:], w1cat[:])
        w2 = const.tile([P, W2 + H], bf16)
        nc.sync.dma_start(w2[:], w2cat[:])

        er1 = big.tile([P, G, H], bf16)
        er2 = big.tile([P, G, H], bf16)
        zs = big.tile([P, G, C], f32)
        ss = big.tile([P, G], f32)

        # ---------------- layer-1 projection: full table (bf16) ----------
        TB = 12   # tiles per DMA batch
        PB = 3    # tiles per PSUM batch
        for t0 in range(0, NTAB // P, TB):
            tb = min(TB, NTAB // P - t0)
            xt = sb.tile([P, TB * P], bf16, tag="xload")
            nc.sync.dma_start(xt[:, :tb * P],
                              xT_all[:, t0 * P:(t0 + tb) * P])
            rows = sb.tile([P, TB, HD + 2 * H], u8, tag="rows")
            for j0 in range(0, tb, PB):
                jb = min(PB, tb - j0)
                pr3 = ps.tile([P, PB, W1], f32, space="PSUM", tag="mm")
                for i in range(jb):
                    nc.tensor.matmul(pr3[:, i, :],
                                     lhsT=xt[:, (j0 + i) * P:(j0 + i + 1) * P],
                                     rhs=w1[:, :W1], start=True, stop=True)
                if (t0 + j0) % 2 == 0:
                    nc.scalar.activation(
                        rows[:, j0:j0 + jb, :HD].bitcast(fp8),
                        pr3[:, :jb, :HD],
                        mybir.ActivationFunctionType.Copy)
                else:
                    nc.vector.tensor_copy(
                        rows[:, j0:j0 + jb, :HD].bitcast(fp8),
                        pr3[:, :jb, :HD])
                nc.vector.tensor_copy(
                    rows[:, j0:j0 + jb, HD:].bitcast(bf16),
                    pr3[:, :jb, HD:W1])
            nc.sync.dma_start(
                tab1[t0 * P:(t0 + tb) * P, :HD + 2 * H].rearrange(
                    "(t p) w -> p t w", p=P), rows[:, :tb, :])

        # er for own dst windows from xT_own
        XC = 10
        for g0 in range(0, G, XC):
            gb = min(XC, G - g0)
            xTo = sb.tile([P, XC * P], bf16, tag="xto", bufs=2)
            nc.sync.dma_start(xTo[:, :gb * P],
                              xT_own[:, g0 * P:(g0 + gb) * P])
            pre = ps.tile([P, XC, H], f32, space="PSUM", tag="mm")
            for i in range(gb):
                nc.tensor.matmul(pre[:, i, :], lhsT=xTo[:, i * P:(i + 1) * P],
                                 rhs=w1[:, W1:W1 + H], start=True, stop=True)
            nc.vector.tensor_copy(er1[:, g0:g0 + gb, :], pre[:, :gb, :])

        # ---------------- edge phase ----------------
        sched_t = [None]

        def load_sched(g):
            if g % SB == 0:
                gb = min(SB, G - g)
                st = sb.tile([P, SB, SCHED_W], i16, tag="sched", bufs=2)
                nc.sync.dma_start(
                    st[:, :gb, :],
                    sched_in[g:g + gb].rearrange("g p w -> p g w"))
                sched_t[0] = st
            return sched_t[0][:, g % SB, :]

        def edge_chunks(layer, suf, g, c0, c1, out_cb):
            """Process chunks [c0, c1) of group g into a PSUM seg tile."""
            k = kprof[g]
            cs = _cs_of(k)
            kn = c1 - c0
            km = KMAX if layer == 1 else KL2
            st = load_sched(g)
            dcol = st[:, KMAX * 8:].bitcast(bf16)
            drep = sb.tile([P, km * P], bf16, tag=f"drep{suf}", bufs=3)
            nc.sync.dma_start(
                drep[:, :kn * P],
                dstl_row_in[g:g + 1, c0 * P:c1 * P].to_broadcast([P, kn * P]))

            if layer == 1:
                erw = er1[:, g, :]
                width, msg, gelem = W1, HD, ELEM1
                tlo, thi = tab1[:32768, :], tab1[HI_OFF:, :]
            else:
                erw = er2[:, g, :]
                width, msg, gelem = W2, HC, ELEM2
                tlo, thi = tab2[:S_LO, :], tab2[HI_OFF:, :]
            gt = gat.tile([P, km, gelem], u8, tag=f"gt{suf}")
            if c0 < cs:
                nlo = (min(c1, cs) - c0) * P
                nc.gpsimd.dma_gather(
                    out_ap=gt[:, :min(c1, cs) - c0, :], in_ap=tlo,
                    idxs_ap=st[:, c0 * 8:min(c1, cs) * 8], num_idxs=nlo,
                    num_idxs_reg=nlo, elem_size=gelem,
                    single_packet=False, queue_num=1)
            if c1 > cs:
                h0 = max(c0, cs)
                nhi = (c1 - h0) * P
                nc.gpsimd.dma_gather(
                    out_ap=gt[:, h0 - c0:c1 - c0, :], in_ap=thi,
                    idxs_ap=st[:, h0 * 8:c1 * 8], num_idxs=nhi,
                    num_idxs_reg=nhi, elem_size=gelem,
                    single_packet=False, queue_num=1)

            fsb = sb.tile([P, km, msg], bf16, tag=f"fsb{suf}", bufs=3)
            nc.scalar.activation(fsb[:, :kn, :],
                                 gt[:, :kn, :msg].bitcast(fp8),
                                 mybir.ActivationFunctionType.Copy)
            fs_ap = fsb[:, :kn, :]
            el_ap = gt[:, :kn, msg:msg + 2 * H].bitcast(bf16)

            # ohT[m, c, j] = (dstl[c*128+j] == m)
            ohT = sb.tile([P, km, P], bf16, tag=f"ohT{suf}", bufs=3)
            nc.vector.tensor_scalar(
                out=ohT[:, :kn, :],
                in0=drep[:, :kn * P].rearrange("p (c j) -> p c j", c=kn),
                scalar1=iotac[:, :1], scalar2=None,
                op0=mybir.AluOpType.is_equal)

            # erp[j, c, h] = sum_m ohT[m,c,j] * erw[m,h]  (+ el via identity)
            erp = psg.tile([P, KMAX, H], f32, space="PSUM", tag="erp")
            for c in range(kn):
                nc.tensor.matmul(erp[:, c, :], lhsT=ohT[:, c, :],
                                 rhs=erw, start=True, stop=False)
            nc.tensor.matmul(erp[:, :kn, :], lhsT=ident[:], rhs=el_ap,
                             start=False, stop=True, skip_group_check=True)

            # ee = exp(lrelu(el + er)) = max(exp(x), exp(0.2 x));
            # mt = [ee*fs | ee]
            ev = sb.tile([P, km, H], bf16, tag=f"ev{suf}")
            nc.scalar.activation(ev[:, :kn, :], erp[:, :kn, :],
                                 mybir.ActivationFunctionType.Exp,
                                 scale=NEG_SLOPE)
            mt = sb.tile([P, km, width], bf16, tag=f"mt{suf}", bufs=3)
            nc.scalar.activation(mt[:, :kn, msg:width], erp[:, :kn, :],
                                 mybir.ActivationFunctionType.Exp)
            nc.vector.tensor_tensor(
                out=mt[:, :kn, msg:width], in0=mt[:, :kn, msg:width],
                in1=ev[:, :kn, :], op=mybir.AluOpType.max)
            nc.vector.tensor_tensor(
                out=mt[:, :kn, :msg].rearrange(
                    "p k (d h) -> p k d h", h=H),
                in0=fs_ap.rearrange("p k (d h) -> p k d h", h=H),
                in1=mt[:, :kn, msg:width, None].rearrange(
                    "p k w o -> p k o w").to_broadcast(
                        [P, kn, msg // H, H]),
                op=mybir.AluOpType.mult)

            # oh_mc[j, m, c] = (dstl[c] == m)
            oh_mc = sb.tile([P, P, km], bf16, tag=f"oh{suf}", bufs=2)
            nc.vector.tensor_tensor(
                out=oh_mc[:, :, :kn],
                in0=dcol[:, None, c0:c1].to_broadcast([P, P, kn]),
                in1=iota_mc[:].rearrange(
                    "p (m c) -> p m c", c=KMAX)[:, :, :kn],
                op=mybir.AluOpType.is_equal)
            # seg[m, w] += sum_j oh_c[j, m] * mt[j, c, w]
            seg = psg.tile([P, W2], f32, space="PSUM", tag="seg")
            for c in range(kn):
                nc.tensor.matmul(seg[:, :width], lhsT=oh_mc[:, :, c],
                                 rhs=mt[:, c, :],
                                 start=(c == 0), stop=(c == kn - 1))
            out_cb(g, seg)

        # ---------------- layer-1 epilogue + inline proj2 ----------------
        segbuf = big.tile([P, G, W2], bf16)

        def l1_out(g, seg):
            dn = sb.tile([P, H], f32, tag="dn")
            nc.vector.tensor_scalar_max(dn[:], seg[:, HD:HD + H], 1e-30)
            rd = sb.tile([P, H], f32, tag="rd")
            nc.vector.reciprocal(rd[:], dn[:])
            ht = sb.tile([P, F_IN], f32, tag="ht")
            nc.vector.tensor_tensor(
                out=ht[:].rearrange("p (d h) -> p d h", h=H),
                in0=seg[:, :HD].rearrange("p (d h) -> p d h", h=H),
                in1=rd[:, None, :].to_broadcast([P, D, H]),
                op=mybir.AluOpType.mult)
            nc.vector.tensor_tensor(
                out=ht[:], in0=ht[:], in1=b1s[:], op=mybir.AluOpType.add)
            mn = sb.tile([P, F_IN], f32, tag="mn")
            nc.vector.tensor_scalar_min(mn[:], ht[:], 0.0)
            nc.scalar.activation(mn[:], mn[:], mybir.ActivationFunctionType.Exp)
            h1g = sb.tile([P, F_IN], bf16, tag="h1g")
            nc.vector.scalar_tensor_tensor(
                out=h1g[:], in0=mn[:], scalar=-1.0, in1=ht[:],
                op0=mybir.AluOpType.add, op1=mybir.AluOpType.max)

            # inline layer-2 projection of this dst window
            hT_ps = ps.tile([F_IN, P], bf16, space="PSUM", tag="mm")
            nc.tensor.transpose(hT_ps[:], h1g[:], ident[:])
            hT = sb.tile([F_IN, P], bf16, tag="hT")
            nc.vector.tensor_copy(hT[:], hT_ps[:])
            pr2 = ps.tile([P, W2 + H], f32, space="PSUM", tag="mm")
            nc.tensor.matmul(pr2[:], lhsT=hT[:], rhs=w2[:],
                             start=True, stop=True)
            nc.vector.tensor_copy(er2[:, g, :], pr2[:, W2:W2 + H])
            row2 = sb.tile([P, CROW], u8, tag="row2")
            nc.scalar.activation(row2[:, :HC].bitcast(fp8), pr2[:, :HC],
                                 mybir.ActivationFunctionType.Copy)
            nc.vector.tensor_copy(row2[:, HC:HC + 2 * H].bitcast(bf16),
                                  pr2[:, HC:HC + H])
            for k, (t0, t1) in enumerate(AG_CHUNKS):
                if g < t1:
                    g2 = g - t0
                    nc.sync.dma_start(
                        tab2_own[k][g2 * P:(g2 + 1) * P, :CROW], row2[:])
                    break

        g_done = 0
        base = 0
        sect = []
        for k, (t0, t1) in enumerate(AG_CHUNKS):
            gk = min(t1, G)
            for g in range(g_done, gk):
                edge_chunks(1, "1", g, 0, kprof[g], l1_out)
            g_done = gk
            rows_k = NCORES * (t1 - t0) * P
            nc.gpsimd.collective_compute(
                "AllGather", mybir.AluOpType.bypass,
                replica_groups=[list(range(NCORES))],
                ins=[tab2_own[k][:]],
                outs=[tab2[base:base + rows_k, :]])
            sect.append((base, base + rows_k))
            base += rows_k



        # ---------------- layer-2 epilogue ----------------
        def l2a_out(g, seg):
            nc.scalar.activation(segbuf[:, g, :], seg[:],
                                 mybir.ActivationFunctionType.Copy)

        def l2_out(g, seg_ps):
            seg = sb.tile([P, W2], f32, tag="segf")
            nc.vector.tensor_tensor(out=seg[:], in0=seg_ps[:],
                                    in1=segbuf[:, g, :],
                                    op=mybir.AluOpType.add)
            dn = sb.tile([P, H], f32, tag="dn2")
            nc.vector.tensor_scalar(
                out=dn[:], in0=seg[:, HC:HC + H], scalar1=1e-30,
                scalar2=float(H), op0=mybir.AluOpType.max,
                op1=mybir.AluOpType.mult)
            rd = sb.tile([P, H], f32, tag="rd2")
            nc.vector.reciprocal(rd[:], dn[:])
            z = sb.tile([P, HC], f32, tag="z")
            nc.vector.tensor_tensor(
                out=z[:].rearrange("p (c h) -> p c h", h=H),
                in0=seg[:, :HC].rearrange("p (c h) -> p c h", h=H),
                in1=rd[:, None, :].to_broadcast([P, C, H]),
                op=mybir.AluOpType.mult)
            z4 = zs[:, g, :]
            nc.vector.reduce_sum(
                z4, z[:].rearrange("p (c h) -> p c h", h=H),
                axis=mybir.AxisListType.X)
            nc.vector.tensor_tensor(
                out=z4, in0=z4, in1=b2ms[:], op=mybir.AluOpType.add)
            es = sb.tile([P, C], f32, tag="es")
            nc.scalar.activation(es[:], z4,
                                 mybir.ActivationFunctionType.Exp,
                                 accum_out=ss[:, g:g + 1])

        tc.no_sync_barrier()
        sched_t[0] = None
        for g in range(G):
            edge_chunks(2, "2", g, 0, _cs_of(kprof[g]), l2a_out)

        tc.no_sync_barrier()
        sched_t[0] = None
        for g in range(G):
            edge_chunks(2, "2", g, _cs_of(kprof[g]), kprof[g], l2_out)

        lg = sb.tile([P, G], f32, tag="lg")
        nc.scalar.activation(lg[:], ss[:], mybir.ActivationFunctionType.Ln)
        nc.vector.tensor_tensor(
            out=zs[:], in0=zs[:],
            in1=lg[:, :, None].to_broadcast([P, G, C]),
            op=mybir.AluOpType.subtract)
        YB = 7
        for g0 in range(0, G, YB):
            gb = min(YB, G - g0)
            nc.sync.dma_start(
                y_out[g0 * P:(g0 + gb) * P, :].rearrange(
                    "(t p) w -> p t w", p=P), zs[:, g0:g0 + gb, :])

    nc.compile()
    return nc


def _perm(h, dim):
    # new position d*h_total + hh  <- old position hh*dim + d
    p = np.empty(h * dim, dtype=np.int64)
    for hh in range(h):
        for d in range(dim):
            p[d * h + hh] = hh * dim + d
    return p


def prepare(x, src, dst, W1s, W1d, al1, ar1, b1, W2s, W2d, al2, ar2, b2):
    """Host prep: schedule, constants, program build. Returns (nc, in_maps)."""
    global _LAST_SCHED
    x = np.asarray(x, dtype=np.float32)
    src = np.asarray(src, dtype=np.int32)
    dst = np.asarray(dst, dtype=np.int32)

    sch = _schedule(src, dst)
    _LAST_SCHED = sch

    perm1 = _perm(H, D)   # 128 -> d-major
    perm2 = _perm(H, C)   # 188 -> c-major

    W1s = np.asarray(W1s, np.float32)
    W1d = np.asarray(W1d, np.float32)
    W2s = np.asarray(W2s, np.float32)
    W2d = np.asarray(W2d, np.float32)
    wel1 = W1s @ _blockdiag(np.asarray(al1, np.float32), HD, H, D)
    wer1 = W1d @ _blockdiag(np.asarray(ar1, np.float32), HD, H, D)
    w1c = np.concatenate([W1s[:, perm1], wel1, wer1], axis=1).astype(BF16)
    wel2 = W2s @ _blockdiag(np.asarray(al2, np.float32), HC, H, C)
    wer2 = W2d @ _blockdiag(np.asarray(ar2, np.float32), HC, H, C)
    w2c = np.concatenate(
        [W2s[:, perm2], wel2, wer2], axis=1)[perm1, :].astype(BF16)

    K = KMAX
    iota_mc = np.tile(np.repeat(np.arange(P, dtype=np.float32), K)[None, :],
                      (P, 1)).astype(BF16)
    iota_c = np.arange(P, dtype=np.float32)[:, None]
    ident_np = np.eye(P, dtype=np.float32).astype(BF16)
    b1_np = np.tile(np.asarray(b1, np.float32)[perm1][None, :], (P, 1))
    b2m_np = np.tile(np.asarray(b2, np.float32).reshape(H, C).mean(0)[None, :],
                     (P, 1))

    # xT in table-row order, replicated; xT_own per core in window order
    xb = x.astype(BF16)
    xT_all = np.zeros((P, NTAB), BF16)
    xT_all[:, sch["rows_all"]] = xb.T
    xT_own = np.zeros((NCORES, P, NPAD), BF16)
    pos = sch["pos_of"]
    for c in range(NCORES):
        xT_own[c][:, pos[c * NPC:(c + 1) * NPC]] = xb[c * NPC:(c + 1) * NPC].T

    nc = _build_program(sch["kprof"])

    in_maps = []
    for c in range(NCORES):
        in_maps.append({
            "xT_all": xT_all,
            "xT_own": xT_own[c],
            "w1cat": w1c, "w2cat": w2c,
            "b1_t": b1_np, "b2m_t": b2m_np,
            "iota_c": iota_c, "ident_t": ident_np,
            "iota_mc": iota_mc,
            "sched_in": sch["sched"][c],
            "dstl_row_in": sch["dstl_row"][c],
        })
    return nc, in_maps


def kernel(x, src, dst, W1s, W1d, al1, ar1, b1, W2s, W2d, al2, ar2, b2):
    global LAST_EXEC_NS, _LAST_NC
    nc, in_maps = prepare(x, src, dst, W1s, W1d, al1, ar1, b1,
                          W2s, W2d, al2, ar2, b2)
    _LAST_NC = nc
    res = run_bass_kernel_spmd(nc, in_maps, list(range(NCORES)))
    LAST_EXEC_NS = res.exec_time_ns
    sch = _LAST_SCHED
    out = np.empty((N, C), np.float32)
    for c in range(NCORES):
        yc = res.results[c]["y_out"]          # [NOUT, C] in window order
        pos = sch["pos_of"][c * NPC:(c + 1) * NPC]
        out[c * NPC:(c + 1) * NPC] = yc[pos]
    return out.astype(np.float32)


# revision 106
# speedup vs baseline: 1.0025x; 1.0025x over previous
"""Two-layer GAT (DGL GATConv) on 8 Trainium2 NeuronCores.

Edge-parallel by dst node with degree-balanced dst windows and a shared
gather schedule for both layers:
  * Core c owns dst nodes [6250c, 6250(c+1)).  Its 6250 nodes are packed
    into 49 windows of 128 so each window's total in-degree fits K_g*128
    slots (K_g = 17 for the first NG17 windows, 15 after) -- ~1% padding
    vs per-half ceils.  Window order/content is a host-side permutation;
    the output is un-permuted on the host.
  * Table rows follow the AllGather chunk layout.  Slots are sorted by
    src table row; the first CS_g chunks are gathered from table[0:32768]
    (idx = row) and the rest from table[18432:] (idx = row - 18432), so
    int16 gather indices reach all 51200 rows with no per-half padding.
    The same index array serves layer 2 (chunks [0, CS_g) are rows
    < 30720 = the AllGather A sections, rest needs B).
  * Layer 1: x replicated; every core projects the FULL table locally in
    bf16 (rows 512B: fs bf16 264B = 128 feats + 4 el) -- no fp8 widen.
  * Layer 2: rows 256B (fs2 fp8 188B c-major | el2 bf16 8B), projected
    inline per group in the layer-1 epilogue and AllGathered in 4 chunks
    that overlap layer-1 edge compute; layer-2 runs as A-pass (chunks
    < CS_g) then B-pass accumulating through a bf16 segbuf.
  * Edge phase per group: dma_gather rows by src, one-hot scatter
    matmuls into PSUM.  er per edge comes from ohT.T @ erw accumulated
    with an identity matmul that adds the gathered el in the same PSUM
    tile; lrelu on DVE, exp on ACT, ee*fs on DVE (2x mode), scatter via
    oh_mc.T @ mt.  d-major feature order keeps broadcasts packed.
Softmax max-subtraction is skipped (|logits| small; alpha shift-invariant).
"""

import math
import os
import sys
from contextlib import ExitStack

import numpy as np

for _p in ("/opt/trn_rl_repo", "/root/.axon_site/_ro/trn_rl_repo"):
    if os.path.isdir(_p) and _p not in sys.path:
        sys.path.append(_p)

import ml_dtypes

import concourse.bass as bass
import concourse.tile as tile
from concourse import bacc, mybir
from concourse.bass_utils import run_bass_kernel_spmd

BF16 = ml_dtypes.bfloat16
FP8 = ml_dtypes.float8_e4m3

N = 50000
E = 800000
F_IN = 128
H, D, C = 4, 32, 47
HD = H * D            # 128
HC = H * C            # 188
NEG_SLOPE = 0.2

NCORES = 8
P = 128
NPC = N // NCORES         # 6250
G = math.ceil(NPC / P)    # 49 dst groups per core
NOUT = G * P              # 6272 output rows per core
NPAD = NOUT               # padded rows per core (one tile per group)
NTAB = NCORES * NPAD      # 50176 table rows

W1 = HD + H               # 132 cols used in layer-1 rows (fs|el)
W2 = HC + H               # 192 (fs2|el2)
ELEM1 = 256               # layer-1 table row bytes (fp8 fs | bf16 el)
ELEM2 = 256               # layer-2 table row bytes (fp8 fs2 | bf16 el2)
OOR = 200.0               # out-of-window dst_local marker
HI_OFF = 18432            # hi-gather table base row
KMAX = 17
K15 = 15
KL2 = 10                  # max chunks per layer-2 pass (= max cs)
CROW = 196                # compact collective row bytes (fs2 fp8|el2 bf16)
S_LO = 32768              # lo-gather table rows

LAST_EXEC_NS = None
_LAST_SCHED = None
_LAST_NC = None

# AllGather chunk boundaries in own-tile units.  The first four cover
# rows 0..33792 > 32768 so the layer-2 A-pass (rows < 32768) can start
# once they have landed; small early chunks keep the collective chain
# abreast of layer-1 edge compute, and the big B chunk transfers while
# the A-pass computes.
AG_CHUNKS = [(0, 8), (8, 16), (16, 25), (25, 33), (33, 49)]
N_ACHUNK = 4              # chunks needed by the layer-2 A-pass


def _row_of(core, pos):
    """Table row of (owning core, padded local position)."""
    conds, vals = [], []
    base = 0
    for (t0, t1) in AG_CHUNKS:
        lo, hi, rpc = t0 * P, t1 * P, (t1 - t0) * P
        conds.append((pos >= lo) & (pos < hi))
        vals.append(base + core * rpc + (pos - lo))
        base += NCORES * rpc
    return np.select(conds, vals)


def _cs_of(k):
    # lo/hi gather split chunk: lo slots hold rows < 32768 (also the
    # layer-2 A-pass, gated on the first three AllGather chunks)
    return 10 if k == KMAX else 8


def _pack_core(deg, ng17):
    """Greedy-pack NPC nodes (by in-degree) into G bins of 128.

    Returns bins [G, 128] of local node ids (-1 = virtual pad)."""
    targets = np.where(np.arange(G) < ng17, KMAX * P, K15 * P)
    order = np.argsort(-deg, kind="stable")
    sums = np.zeros(G, np.int64)
    cards = np.zeros(G, np.int64)
    bins = np.full((G, P), -1, np.int64)
    for v in order:
        d = deg[v]
        # bin whose remaining slots need the highest average degree
        need = (targets - sums - d) / np.maximum(P - cards, 1)
        ok = (cards < P) & (targets - sums >= d)
        if ok.any():
            g = int(np.where(ok, need, -np.inf).argmax())
        else:
            g = int(np.where(cards < P, targets - sums, -np.inf).argmax())
        bins[g, cards[g]] = v
        cards[g] += 1
        sums[g] += d
    if not (sums <= targets).all():
        return None
    return bins


def _wrap(a):
    # [n] gather idx list -> [128, n/16] wrapped+replicated int16
    n = a.shape[-1]
    w = a.reshape(n // 16, 16).T            # [16, n/16]
    return np.tile(w, (8, 1)).astype(np.int16)


def _schedule(src, dst):
    """Degree-balanced windows + straddled lo/hi chunk schedule.

    Returns per-core packed arrays + the shared K profile."""
    src = src.astype(np.int64)
    dst = dst.astype(np.int64)
    core_of = dst // NPC
    dloc = dst % NPC

    deg = np.zeros((NCORES, NPC), np.int64)
    np.add.at(deg, (core_of, dloc), 1)
    tot = deg.sum(1)
    ng17 = int(max(2, math.ceil((tot.max() - G * K15 * P) / ((KMAX - K15) * P))))
    ng17 = min(G, ng17 + 2)   # slack for card-128 capacity stranding
    while True:
        packed = [_pack_core(deg[c], ng17) for c in range(NCORES)]
        if all(b is not None for b in packed):
            break
        assert ng17 < G, "packing failed even with uniform KMAX"
        ng17 = min(G, ng17 + 4)
    # small groups first so the first AllGather fires earlier
    kprof = [K15 if g < G - ng17 else KMAX for g in range(G)]
    bins = np.stack([b[::-1] for b in packed])

    # local node -> (group, window pos); virtual pads keep -1
    g_of = np.zeros((NCORES, NPC), np.int64)
    w_of = np.zeros((NCORES, NPC), np.int64)
    for c in range(NCORES):
        flat = bins[c].ravel()
        real = flat >= 0
        g_of[c, flat[real]] = np.nonzero(real)[0] // P
        w_of[c, flat[real]] = np.nonzero(real)[0] % P

    # permuted padded position of every global node (for table rows)
    pos_of = np.zeros(N, np.int64)
    for c in range(NCORES):
        pos_of[c * NPC:(c + 1) * NPC] = g_of[c] * P + w_of[c]
    rows_all = _row_of(np.arange(N) // NPC, pos_of)

    rowsrc = rows_all[src]
    eg = g_of[core_of, dloc]
    ew = w_of[core_of, dloc]

    KW = KMAX * 8
    idx_w = np.zeros((NCORES, G, P, KW), np.int16)
    dstl_col = np.full((NCORES, G, P, KMAX), OOR, np.float32)
    dstl_row = np.full((NCORES, G, KMAX * P), OOR, np.float32)

    for c in range(NCORES):
        sel = core_of == c
        r, g, w = rowsrc[sel], eg[sel], ew[sel]
        order = np.lexsort((r, g))
        r, g, w = r[order], g[order], w[order]
        starts = np.searchsorted(g, np.arange(G + 1))
        for gg in range(G):
            k = kprof[gg]
            cs = _cs_of(k)
            lo_slots, slots = cs * P, k * P
            rg = r[starts[gg]:starts[gg + 1]]
            wg = w[starts[gg]:starts[gg + 1]]
            cnt = len(rg)
            assert cnt <= slots, (c, gg, cnt, slots)
            n_lo = min(int(np.searchsorted(rg, S_LO)), lo_slots)
            assert int(np.searchsorted(rg, HI_OFF)) <= n_lo, (c, gg)
            assert cnt - n_lo <= slots - lo_slots, (c, gg, cnt, n_lo)
            idx = np.zeros(slots, np.int64)
            dl = np.full(slots, OOR, np.float32)
            idx[:n_lo] = rg[:n_lo]
            dl[:n_lo] = wg[:n_lo]
            n_hi = cnt - n_lo
            idx[lo_slots:lo_slots + n_hi] = rg[n_lo:] - HI_OFF
            dl[lo_slots:lo_slots + n_hi] = wg[n_lo:]
            assert (idx >= 0).all() and (idx < 32768).all()
            idx_w[c, gg, :, :cs * 8] = _wrap(idx[:lo_slots])
            idx_w[c, gg, :, cs * 8:k * 8] = _wrap(idx[lo_slots:])
            dstl_col[c, gg, :, :k] = dl.reshape(k, P).T
            dstl_row[c, gg, :slots] = dl

    sched = np.concatenate(
        [idx_w, dstl_col.astype(BF16).view(np.int16)], axis=3)
    return dict(sched=sched, dstl_row=dstl_row.astype(BF16), kprof=kprof,
                bins=bins, rows_all=rows_all, pos_of=pos_of)


def _blockdiag(a, hd, h, dim):
    out = np.zeros((hd, h), dtype=np.float32)
    for i in range(h):
        out[i * dim:(i + 1) * dim, i] = a[i]
    return out


def _build_program(kprof):
    SCHED_W = KMAX * 8 + KMAX    # idx cols + dstl_col cols (int16)
    SB = 7                       # sched groups per DMA batch
    nc = bacc.Bacc("TRN2", target_bir_lowering=False, debug=False,
                   num_devices=NCORES, num_swdge_queues=2)
    dt = mybir.dt
    f32, bf16, i16, u8 = dt.float32, dt.bfloat16, dt.int16, dt.uint8
    fp8 = dt.float8e4

    def inp(name, shape, d=f32):
        return nc.dram_tensor(name, shape, d, kind="ExternalInput").ap()

    xT_all = inp("xT_all", [P, NTAB], fp8)       # replicated, table-row order
    xT_own = inp("xT_own", [P, NPAD], bf16)      # own nodes, window order
    w1cat = inp("w1cat", [F_IN, W1 + H], bf16)   # fs(d-major)|el|er
    w2cat = inp("w2cat", [F_IN, W2 + H], bf16)   # fs2(c-major)|el2|er2
    b1_t = inp("b1_t", [P, HD])
    b2m_t = inp("b2m_t", [P, C])
    iota_mc_in = inp("iota_mc", [P, P * KMAX], bf16)  # [p, m*KMAX+c] = m
    iota_c = inp("iota_c", [P, 1])               # iota_c[p, 0] = p (f32)
    ident_t = inp("ident_t", [P, P], bf16)
    sched_in = inp("sched_in", [G, P, SCHED_W], i16)
    dstl_row_in = inp("dstl_row_in", [G, KMAX * P], bf16)

    y_out = nc.dram_tensor("y_out", [NOUT, C], f32, kind="ExternalOutput").ap()

    tab1 = nc.dram_tensor("tab1", [NTAB, ELEM1], u8).ap()
    tab2_own = [
        nc.dram_tensor(f"tab2_own{k}", [(t1 - t0) * P, ELEM2], u8).ap()
        for k, (t0, t1) in enumerate(AG_CHUNKS)]
    tab2 = nc.dram_tensor("tab2", [NTAB, ELEM2], u8,
                          addr_space="Shared").ap()

    with tile.TileContext(nc) as tc, ExitStack() as ctx:
        const = ctx.enter_context(tc.tile_pool(name="const", bufs=1))
        sb = ctx.enter_context(tc.tile_pool(name="sb", bufs=3))
        gat = ctx.enter_context(tc.tile_pool(name="gat", bufs=3))
        ps = ctx.enter_context(tc.tile_pool(name="ps", bufs=2, space="PSUM"))
        psg = ctx.enter_context(tc.tile_pool(name="psg", bufs=3, space="PSUM"))
        big = ctx.enter_context(tc.tile_pool(name="big", bufs=1))

        iota_mc = const.tile([P, P * KMAX], bf16)
        nc.sync.dma_start(iota_mc[:], iota_mc_in[:])
        iotac = const.tile([P, 1], f32)
        nc.sync.dma_start(iotac[:], iota_c[:])
        ident = const.tile([P, P], bf16)
        nc.sync.dma_start(ident[:], ident_t[:])
        b1s = const.tile([P, HD], f32)
        nc.sync.dma_start(b1s[:], b1_t[:])
        b2ms = const.tile([P, C], f32)
        nc.sync.dma_start(b2ms[:], b2m_t[:])
        w1 = const.tile([P, W1 + H], bf16)
        nc.sync.dma_start(w1[:], w1cat[:])
        w2 = const.tile([P, W2 + H], bf16)
        nc.sync.dma_start(w2[:], w2cat[:])

        er1 = big.tile([P, G, H], bf16)
        er2 = big.tile([P, G, H], bf16)
        zs = big.tile([P, G, C], f32)
        ss = big.tile([P, G], f32)

        # er for own dst windows from xT_own (first: feeds group-0 edges)
        XC = 10
        for g0 in range(0, G, XC):
            gb = min(XC, G - g0)
            xTo = sb.tile([P, XC * P], bf16, tag="xto", bufs=2)
            nc.sync.dma_start(xTo[:, :gb * P],
                              xT_own[:, g0 * P:(g0 + gb) * P])
            pre = ps.tile([P, XC, H], f32, space="PSUM", tag="mm")
            for i in range(gb):
                nc.tensor.matmul(pre[:, i, :], lhsT=xTo[:, i * P:(i + 1) * P],
                                 rhs=w1[:, W1:W1 + H], start=True, stop=True)
            nc.vector.tensor_copy(er1[:, g0:g0 + gb, :], pre[:, :gb, :])

        # ---------------- layer-1 projection: full table ----------------
        # tab1lo (rows < S_LO) finishes first so lo-gathers overlap the
        # hi-half of the projection; the [HI_OFF, S_LO) overlap is written
        # to both tables.
        TB = 12   # tiles per DMA batch
        PB = 3    # tiles per PSUM batch
        for t0 in range(0, NTAB // P, TB):
            tb = min(TB, NTAB // P - t0)
            xt = sb.tile([P, TB * P], fp8, tag="xload")
            nc.sync.dma_start(xt[:, :tb * P],
                              xT_all[:, t0 * P:(t0 + tb) * P])
            rows = sb.tile([P, TB, HD + 2 * H], u8, tag="rows")
            for j0 in range(0, tb, PB):
                jb = min(PB, tb - j0)
                pr3 = ps.tile([P, PB, W1], f32, space="PSUM", tag="mm")
                for i in range(jb):
                    nc.tensor.matmul(pr3[:, i, :],
                                     lhsT=xt[:, (j0 + i) * P:(j0 + i + 1) * P],
                                     rhs=w1[:, :W1], start=True, stop=True)
                if (t0 + j0) % 2 == 0:
                    nc.scalar.activation(
                        rows[:, j0:j0 + jb, :HD].bitcast(fp8),
                        pr3[:, :jb, :HD],
                        mybir.ActivationFunctionType.Copy)
                else:
                    nc.vector.tensor_copy(
                        rows[:, j0:j0 + jb, :HD].bitcast(fp8),
                        pr3[:, :jb, :HD])
                nc.vector.tensor_copy(
                    rows[:, j0:j0 + jb, HD:].bitcast(bf16),
                    pr3[:, :jb, HD:W1])
            nc.sync.dma_start(
                tab1[t0 * P:(t0 + tb) * P, :HD + 2 * H].rearrange(
                    "(t p) w -> p t w", p=P), rows[:, :tb, :])

        # ---------------- edge phase ----------------
        sched_t = [None]

        def load_sched(g):
            if g % SB == 0:
                gb = min(SB, G - g)
                st = sb.tile([P, SB, SCHED_W], i16, tag="sched", bufs=2)
                nc.sync.dma_start(
                    st[:, :gb, :],
                    sched_in[g:g + gb].rearrange("g p w -> p g w"))
                sched_t[0] = st
            return sched_t[0][:, g % SB, :]

        def edge_chunks(layer, suf, g, c0, c1, out_cb):
            """Process chunks [c0, c1) of group g into a PSUM seg tile."""
            k = kprof[g]
            cs = _cs_of(k)
            kn = c1 - c0
            km = KMAX if layer == 1 else KL2
            st = load_sched(g)
            dcol = st[:, KMAX * 8:].bitcast(bf16)
            drep = sb.tile([P, km * P], bf16, tag=f"drep{suf}", bufs=4)
            nc.sync.dma_start(
                drep[:, :kn * P],
                dstl_row_in[g:g + 1, c0 * P:c1 * P].to_broadcast(
                    [P, kn * P]))
            if layer == 1:
                erw = er1[:, g, :]
                width, msg, gelem = W1, HD, ELEM1
                tlo, thi = tab1[:S_LO, :], tab1[HI_OFF:, :]
            else:
                erw = er2[:, g, :]
                width, msg, gelem = W2, HC, ELEM2
                tlo, thi = tab2[:S_LO, :], tab2[HI_OFF:, :]
                assert NTAB - HI_OFF <= 32768
            gt = gat.tile([P, km, gelem], u8, tag=f"gt{suf}")
            if c0 < cs:
                nlo = (min(c1, cs) - c0) * P
                nc.gpsimd.dma_gather(
                    out_ap=gt[:, :min(c1, cs) - c0, :], in_ap=tlo,
                    idxs_ap=st[:, c0 * 8:min(c1, cs) * 8], num_idxs=nlo,
                    num_idxs_reg=nlo, elem_size=gelem,
                    single_packet=False, queue_num=1)
            if c1 > cs:
                h0 = max(c0, cs)
                nhi = (c1 - h0) * P
                nc.gpsimd.dma_gather(
                    out_ap=gt[:, h0 - c0:c1 - c0, :], in_ap=thi,
                    idxs_ap=st[:, h0 * 8:c1 * 8], num_idxs=nhi,
                    num_idxs_reg=nhi, elem_size=gelem,
                    single_packet=False, queue_num=1)

            fsb = sb.tile([P, km, msg], bf16, tag=f"fsb{suf}", bufs=3)
            nc.scalar.activation(fsb[:, :kn, :],
                                 gt[:, :kn, :msg].bitcast(fp8),
                                 mybir.ActivationFunctionType.Copy)
            fs_ap = fsb[:, :kn, :]
            el_ap = gt[:, :kn, msg:msg + 2 * H].bitcast(bf16)

            mt = sb.tile([P, km, width], bf16, tag=f"mt{suf}", bufs=4)
            ev = sb.tile([P, km, H], bf16, tag=f"ev{suf}")
            # ohT[m, c, j] = (dstl[c*128+j] == m)
            ohT = sb.tile([P, km, P], bf16, tag=f"ohT{suf}", bufs=3)
            nc.vector.tensor_scalar(
                out=ohT[:, :kn, :],
                in0=drep[:, :kn * P].rearrange("p (c j) -> p c j", c=kn),
                scalar1=iotac[:, :1], scalar2=None,
                op0=mybir.AluOpType.is_equal)

            # erp[j, c, h] = sum_m ohT[m,c,j]*erw[m,h]  (+ el, identity)
            erp = psg.tile([P, KMAX, H], f32, space="PSUM", tag="erp")
            for c in range(kn):
                nc.tensor.matmul(erp[:, c, :], lhsT=ohT[:, c, :],
                                 rhs=erw, start=True, stop=False)
            nc.tensor.matmul(erp[:, :kn, :], lhsT=ident[:], rhs=el_ap,
                             start=False, stop=True,
                             skip_group_check=True)
            # ee = exp(lrelu(el + er)) = max(exp(x), exp(0.2 x))
            nc.scalar.activation(ev[:, :kn, :], erp[:, :kn, :],
                                 mybir.ActivationFunctionType.Exp,
                                 scale=NEG_SLOPE)
            nc.scalar.activation(mt[:, :kn, msg:width], erp[:, :kn, :],
                                 mybir.ActivationFunctionType.Exp)
            nc.vector.tensor_tensor(
                out=mt[:, :kn, msg:width], in0=mt[:, :kn, msg:width],
                in1=ev[:, :kn, :], op=mybir.AluOpType.max)
            nc.vector.tensor_tensor(
                out=mt[:, :kn, :msg].rearrange(
                    "p k (d h) -> p k d h", h=H),
                in0=fs_ap.rearrange("p k (d h) -> p k d h", h=H),
                in1=mt[:, :kn, msg:width, None].rearrange(
                    "p k w o -> p k o w").to_broadcast(
                        [P, kn, msg // H, H]),
                op=mybir.AluOpType.mult)

            # oh_mc[j, m, c] = (dstl[c] == m)
            oh_mc = sb.tile([P, P, km], bf16, tag=f"oh{suf}", bufs=2)
            nc.vector.tensor_tensor(
                out=oh_mc[:, :, :kn],
                in0=dcol[:, None, c0:c1].to_broadcast([P, P, kn]),
                in1=iota_mc[:].rearrange(
                    "p (m c) -> p m c", c=KMAX)[:, :, :kn],
                op=mybir.AluOpType.is_equal)
            # seg[m, w] += sum_j oh_c[j, m] * mt[j, c, w]
            seg = psg.tile([P, W2], f32, space="PSUM", tag="seg")
            for c in range(kn):
                nc.tensor.matmul(seg[:, :width], lhsT=oh_mc[:, :, c],
                                 rhs=mt[:, c, :],
                                 start=(c == 0), stop=(c == kn - 1))
            out_cb(g, seg, ohT, erp, kn)

        # ---------------- layer-1 epilogue + inline proj2 ----------------
        segbuf = big.tile([P, G, W2], bf16)

        def l1_out(g, seg, ohT, erp, kn):
            dn = sb.tile([P, H], f32, tag="dn")
            nc.vector.tensor_scalar_max(dn[:], seg[:, HD:HD + H], 1e-30)
            rd = sb.tile([P, H], f32, tag="rd")
            nc.vector.reciprocal(rd[:], dn[:])
            ht = sb.tile([P, F_IN], f32, tag="ht")
            nc.vector.tensor_tensor(
                out=ht[:].rearrange("p (d h) -> p d h", h=H),
                in0=seg[:, :HD].rearrange("p (d h) -> p d h", h=H),
                in1=rd[:, None, :].to_broadcast([P, D, H]),
                op=mybir.AluOpType.mult)
            nc.vector.tensor_tensor(
                out=ht[:], in0=ht[:], in1=b1s[:], op=mybir.AluOpType.add)
            mn = sb.tile([P, F_IN], f32, tag="mn")
            nc.vector.tensor_scalar_min(mn[:], ht[:], 0.0)
            nc.scalar.activation(mn[:], mn[:], mybir.ActivationFunctionType.Exp)
            h1g = sb.tile([P, F_IN], bf16, tag="h1g")
            nc.vector.scalar_tensor_tensor(
                out=h1g[:], in0=mn[:], scalar=-1.0, in1=ht[:],
                op0=mybir.AluOpType.add, op1=mybir.AluOpType.max)

            # inline layer-2 projection of this dst window
            hT_ps = ps.tile([F_IN, P], bf16, space="PSUM", tag="mm")
            nc.tensor.transpose(hT_ps[:], h1g[:], ident[:])
            hT = sb.tile([F_IN, P], bf16, tag="hT")
            nc.vector.tensor_copy(hT[:], hT_ps[:])
            pr2 = ps.tile([P, W2 + H], f32, space="PSUM", tag="mm")
            nc.tensor.matmul(pr2[:], lhsT=hT[:], rhs=w2[:],
                             start=True, stop=True)
            nc.vector.tensor_copy(er2[:, g, :], pr2[:, W2:W2 + H])
            row2 = sb.tile([P, CROW], u8, tag="row2")
            nc.scalar.activation(row2[:, :HC].bitcast(fp8), pr2[:, :HC],
                                 mybir.ActivationFunctionType.Copy)
            nc.vector.tensor_copy(row2[:, HC:HC + 2 * H].bitcast(bf16),
                                  pr2[:, HC:HC + H])
            for k, (t0, t1) in enumerate(AG_CHUNKS):
                if g < t1:
                    g2 = g - t0
                    nc.sync.dma_start(
                        tab2_own[k][g2 * P:(g2 + 1) * P, :CROW], row2[:])
                    break

        g_done = 0
        base = 0
        sect = []
        for k, (t0, t1) in enumerate(AG_CHUNKS):
            gk = min(t1, G)
            for g in range(g_done, gk):
                edge_chunks(1, "1", g, 0, kprof[g], l1_out)
            g_done = gk
            rows_k = NCORES * (t1 - t0) * P
            nc.gpsimd.collective_compute(
                "AllGather", mybir.AluOpType.bypass,
                replica_groups=[list(range(NCORES))],
                ins=[tab2_own[k][:]],
                outs=[tab2[base:base + rows_k, :]])
            sect.append((base, base + rows_k))
            base += rows_k



        # ---------------- layer-2 epilogue ----------------
        def l2a_out(g, seg, ohT, erp, kn):
            nc.scalar.activation(segbuf[:, g, :], seg[:],
                                 mybir.ActivationFunctionType.Copy)

        def l2_out(g, seg_ps, ohT, erp, kn):
            # accumulate into segbuf; softmax math is batched per FB groups
            nc.vector.tensor_tensor(out=segbuf[:, g, :], in0=seg_ps[:],
                                    in1=segbuf[:, g, :],
                                    op=mybir.AluOpType.add)

        FB = 5    # groups per softmax batch

        def softmax_batch(g0, gb):
            sbv = segbuf[:, g0:g0 + gb, :]
            dn = sb.tile([P, FB, H], f32, tag="dn2", bufs=2)
            nc.vector.tensor_scalar(
                out=dn[:, :gb, :], in0=sbv[:, :, HC:HC + H], scalar1=1e-30,
                scalar2=float(H), op0=mybir.AluOpType.max,
                op1=mybir.AluOpType.mult)
            rd = sb.tile([P, FB, H], f32, tag="rd2", bufs=2)
            nc.vector.reciprocal(rd[:, :gb, :], dn[:, :gb, :])
            z = sb.tile([P, FB, HC], f32, tag="z", bufs=2)
            nc.gpsimd.tensor_tensor(
                out=z[:, :gb, :].rearrange("p g (c h) -> p g c h", h=H),
                in0=sbv[:, :, :HC].rearrange("p g (c h) -> p g c h", h=H),
                in1=rd[:, :gb, None, :].to_broadcast([P, gb, C, H]),
                op=mybir.AluOpType.mult)
            z4 = zs[:, g0:g0 + gb, :]
            nc.vector.reduce_sum(
                z4, z[:, :gb, :].rearrange("p g (c h) -> p g c h", h=H),
                axis=mybir.AxisListType.X)
            nc.vector.tensor_tensor(
                out=z4, in0=z4,
                in1=b2ms[:, None, :].to_broadcast([P, gb, C]),
                op=mybir.AluOpType.add)
            es = sb.tile([P, FB, C], f32, tag="es", bufs=2)
            nc.scalar.activation(es[:, :gb, :], z4,
                                 mybir.ActivationFunctionType.Exp)
            nc.vector.reduce_sum(
                ss[:, g0:g0 + gb], es[:, :gb, :],
                axis=mybir.AxisListType.X)

        tc.no_sync_barrier()
        sched_t[0] = None
        for g in range(G):
            edge_chunks(2, "2", g, 0, _cs_of(kprof[g]), l2a_out)

        tc.no_sync_barrier()
        sched_t[0] = None
        for g in range(G):
            edge_chunks(2, "2", g, _cs_of(kprof[g]), kprof[g], l2_out)
            if (g + 1) % FB == 0 or g == G - 1:
                g0 = (g // FB) * FB
                softmax_batch(g0, g - g0 + 1)

        lg = sb.tile([P, G], f32, tag="lg")
        nc.scalar.activation(lg[:], ss[:], mybir.ActivationFunctionType.Ln)
        nc.vector.tensor_tensor(
            out=zs[:], in0=zs[:],
            in1=lg[:, :, None].to_broadcast([P, G, C]),
            op=mybir.AluOpType.subtract)
        YB = 7
        for g0 in range(0, G, YB):
            gb = min(YB, G - g0)
            nc.sync.dma_start(
                y_out[g0 * P:(g0 + gb) * P, :].rearrange(
                    "(t p) w -> p t w", p=P), zs[:, g0:g0 + gb, :])

    nc.compile()
    return nc


def _perm(h, dim):
    # new position d*h_total + hh  <- old position hh*dim + d
    p = np.empty(h * dim, dtype=np.int64)
    for hh in range(h):
        for d in range(dim):
            p[d * h + hh] = hh * dim + d
    return p


def prepare(x, src, dst, W1s, W1d, al1, ar1, b1, W2s, W2d, al2, ar2, b2):
    """Host prep: schedule, constants, program build. Returns (nc, in_maps)."""
    global _LAST_SCHED
    x = np.asarray(x, dtype=np.float32)
    src = np.asarray(src, dtype=np.int32)
    dst = np.asarray(dst, dtype=np.int32)

    sch = _schedule(src, dst)
    _LAST_SCHED = sch

    perm1 = _perm(H, D)   # 128 -> d-major
    perm2 = _perm(H, C)   # 188 -> c-major

    W1s = np.asarray(W1s, np.float32)
    W1d = np.asarray(W1d, np.float32)
    W2s = np.asarray(W2s, np.float32)
    W2d = np.asarray(W2d, np.float32)
    wel1 = W1s @ _blockdiag(np.asarray(al1, np.float32), HD, H, D)
    wer1 = W1d @ _blockdiag(np.asarray(ar1, np.float32), HD, H, D)
    w1c = np.concatenate([W1s[:, perm1], wel1, wer1], axis=1).astype(BF16)
    wel2 = W2s @ _blockdiag(np.asarray(al2, np.float32), HC, H, C)
    wer2 = W2d @ _blockdiag(np.asarray(ar2, np.float32), HC, H, C)
    w2c = np.concatenate(
        [W2s[:, perm2], wel2, wer2], axis=1)[perm1, :].astype(BF16)

    K = KMAX
    iota_mc = np.tile(np.repeat(np.arange(P, dtype=np.float32), K)[None, :],
                      (P, 1)).astype(BF16)
    iota_c = np.arange(P, dtype=np.float32)[:, None]
    ident_np = np.eye(P, dtype=np.float32).astype(BF16)
    b1_np = np.tile(np.asarray(b1, np.float32)[perm1][None, :], (P, 1))
    b2m_np = np.tile(np.asarray(b2, np.float32).reshape(H, C).mean(0)[None, :],
                     (P, 1))

    # xT in table-row order, replicated; xT_own per core in window order
    xb = x.astype(BF16)
    xT_all = np.zeros((P, NTAB), BF16)
    xT_all[:, sch["rows_all"]] = xb.T
    xT_own = np.zeros((NCORES, P, NPAD), BF16)
    pos = sch["pos_of"]
    for c in range(NCORES):
        xT_own[c][:, pos[c * NPC:(c + 1) * NPC]] = xb[c * NPC:(c + 1) * NPC].T

    nc = _build_program(sch["kprof"])

    in_maps = []
    for c in range(NCORES):
        in_maps.append({
            "xT_all": xT_all,
            "xT_own": xT_own[c],
            "w1cat": w1c, "w2cat": w2c,
            "b1_t": b1_np, "b2m_t": b2m_np,
            "iota_c": iota_c, "ident_t": ident_np,
            "iota_mc": iota_mc,
            "sched_in": sch["sched"][c],
            "dstl_row_in": sch["dstl_row"][c],
        })
    return nc, in_maps


def kernel(x, src, dst, W1s, W1d, al1, ar1, b1, W2s, W2d, al2, ar2, b2):
    global LAST_EXEC_NS, _LAST_NC
    nc, in_maps = prepare(x, src, dst, W1s, W1d, al1, ar1, b1,
                          W2s, W2d, al2, ar2, b2)
    _LAST_NC = nc
    res = run_bass_kernel_spmd(nc, in_maps, list(range(NCORES)))
    LAST_EXEC_NS = res.exec_time_ns
    sch = _LAST_SCHED
    out = np.empty((N, C), np.float32)
    for c in range(NCORES):
        yc = res.results[c]["y_out"]          # [NOUT, C] in window order
        pos = sch["pos_of"][c * NPC:(c + 1) * NPC]
        out[c * NPC:(c + 1) * NPC] = yc[pos]
    return out.astype(np.float32)
